# revision 13
# baseline (speedup 1.0000x reference)
"""RNN-T Joiner kernel for 8x TRN2 NeuronCores (Bass/Tile).

out[b,t,u,v] = (enc[b,t]@W_enc.T + b_enc) @ W1.T
            + (pred[b,u]@W_pred.T + b_pred) @ W2.T + b_out
with W1 = W_out[:, :J], W2 = W_out[:, J:].

Strategy: data-parallel over batch (B=8 == n_cores). Host folds the two
back-to-back projections into single matrices (associativity):
  Ev = enc @ (W1@W_enc).T        [T, V]
  Pb = pred @ (W2@W_pred).T + c  [U, V],  c = W1@b_enc + W2@b_pred + b_out
so the device does one GEMM stage instead of two. Per core (one batch):
  S2: Ev (2 t-blocks of 128) and Pb via PE matmuls into PSUM, ACT copies
      to bf16 SBUF. Inputs host-transposed and pre-packed to [128, X] so
      each tensor is a single contiguous DMA.
  S3 (the output): out[t, u, :] = Ev[t, :] + Pb[u, :].
      First UBLK u-values (per t-block) go through the PE: selector-matmul
      broadcasts Pb[u] into PSUM, identity-matmul accumulates Ev, ACT
      copies PSUM -> bf16 out tile. This fills the pipe while Pbrep builds.
      Remaining u: Pb rows are pre-broadcast ("Pbrep" groups of UBLK u,
      double-buffered: sel-matmul -> PSUM -> ACT copy -> bf16 SBUF); the
      add is then ONE pure-SBUF bf16 DVE tensor_tensor per out tile
      (FD=4096, in0 = Ev repeated via a stride-0 broadcast AP, 2x mode).
All matmul operands are bf16 (1 cyc/col on the PE, FWL weight loads);
accumulation stays fp32 in PSUM. Output is written to HBM as bf16 (halves
write traffic; tolerance is 2e-2) and upcast to f32 on host.
"""

import numpy as np

ENC_DIM, DEC_DIM, J, V = 512, 640, 512, 1024
B, T, U = 8, 256, 64
N_CORES = 8
UBLK = 8  # u-values per output tile / DMA ([128, UBLK*1024] bf16 = 2MB DMA)
NG = U // UBLK  # 8 u-groups

_CACHE: dict = {}


def _ensure_path():
    try:
        import concourse.bass  # noqa: F401
    except ImportError:
        import sys

        for p in ("/opt/trn_rl_repo", "/root/.axon_site/_ro/trn_rl_repo"):
            if p not in sys.path:
                sys.path.insert(0, p)


def _build_nc():
    import concourse.mybir as mybir
    from concourse import bacc
    from concourse.masks import make_identity
    from concourse.tile import TileContext

    f32 = mybir.dt.float32
    bf16 = mybir.dt.bfloat16
    nc = bacc.Bacc("TRN2", target_bir_lowering=False, debug=False,
                   num_devices=N_CORES)

    NE = ENC_DIM // 128  # 4 contraction chunks for Ev
    ND = DEC_DIM // 128  # 5 contraction chunks for Pb
    NT = T // 128   # 2 t-blocks
    NV = V // 512   # 2 psum-bank v-chunks

    # All inputs host-packed to [128, nchunks*cols] (partition-major chunks).
    encT_d = nc.dram_tensor("encT", [128, NE * T], bf16, kind="ExternalInput")
    predT_d = nc.dram_tensor("predT", [128, ND * U], bf16, kind="ExternalInput")
    wce_d = nc.dram_tensor("wceT", [128, NE * V], bf16, kind="ExternalInput")
    wcp_d = nc.dram_tensor("wcpT", [128, ND * V], bf16, kind="ExternalInput")
    cvec_d = nc.dram_tensor("cvec", [1, V], bf16, kind="ExternalInput")
    out_d = nc.dram_tensor("out", [T, U * V], bf16, kind="ExternalOutput")

    with TileContext(nc) as tc:
        with (
            tc.tile_pool(name="const", bufs=1) as const,
            tc.tile_pool(name="pbrep", bufs=3) as pbpool,
            tc.tile_pool(name="otile", bufs=4) as opool,
            tc.tile_pool(name="ps", bufs=4, space="PSUM") as psp,
        ):
            def load(tag, dram, cols):
                t = const.tile([128, cols], bf16, tag=tag, name=tag)
                nc.sync.dma_start(t[:, :], dram.ap()[:, :])
                return t

            preds_a = load("pred", predT_d, ND * U)
            wcp_a = load("wcp", wcp_d, ND * V)
            encs_a = load("enc", encT_d, NE * T)
            wce_a = load("wce", wce_d, NE * V)
            cvec = const.tile([1, V], bf16, tag="cvec", name="cvec")
            nc.sync.dma_start(cvec[:, :], cvec_d.ap()[:, :])

            ident = const.tile([128, 128], bf16, tag="ident", name="ident")
            make_identity(nc, ident[:, :])
            ones = const.tile([1, 128], bf16, tag="ones", name="ones")
            nc.gpsimd.memset(ones[:, :], 1.0)
            # sel[k, u*128+m] = 1 if k == u else 0: sel[:, u*128:(u+1)*128] is
            # the lhsT that broadcasts Pb row u across all 128 out partitions.
            sel = const.tile([U, U * 128], bf16, tag="sel", name="sel")
            nc.gpsimd.memset(sel[:, :], 0.0)
            nc.gpsimd.affine_select(
                out=sel[:, :].rearrange("p (u m) -> p u m", m=128),
                in_=sel[:, :].rearrange("p (u m) -> p u m", m=128),
                compare_op=mybir.AluOpType.not_equal,
                fill=1.0,
                base=0,
                pattern=[[-1, U], [0, 128]],
                channel_multiplier=1,
            )

            # S2b: Pb = pred @ Wcp.T + c -> [64, 1024]  (c via K=1 matmul)
            Pb = const.tile([U, V], bf16, tag="Pb", name="Pb")
            for vb in range(NV):
                ps = psp.tile([128, V], f32, tag="ps", name="ps")
                for c in range(ND):
                    nc.tensor.matmul(ps[:U, :512],
                                     lhsT=preds_a[:, c * U:(c + 1) * U],
                                     rhs=wcp_a[:, c * V + vb * 512:c * V + (vb + 1) * 512],
                                     start=(c == 0), stop=False)
                nc.tensor.matmul(ps[:U, :512], lhsT=ones[:, :U],
                                 rhs=cvec[:, vb * 512:(vb + 1) * 512],
                                 start=False, stop=True)
                nc.scalar.copy(Pb[:, vb * 512:(vb + 1) * 512], ps[:U, :512])

            # S2a: Ev[tb] = enc @ Wce.T -> [128, 1024] per t-block
            Ev = [const.tile([128, V], bf16, tag=f"Ev{tb}", name=f"Ev{tb}") for tb in range(NT)]
            for tb in range(NT):
                for vb in range(NV):
                    ps = psp.tile([128, V], f32, tag="ps", name="ps")
                    for c in range(NE):
                        nc.tensor.matmul(
                            ps[:, :512],
                            lhsT=encs_a[:, c * T + tb * 128:c * T + (tb + 1) * 128],
                            rhs=wce_a[:, c * V + vb * 512:c * V + (vb + 1) * 512],
                            start=(c == 0), stop=(c == NE - 1))
                    nc.scalar.copy(Ev[tb][:, vb * 512:(vb + 1) * 512], ps[:, :512])

            def build_pbrep(g):
                """Pre-broadcast Pb rows u=g*UBLK..g*UBLK+UBLK-1 across all
                128 partitions: [128, UBLK*1024] bf16."""
                rep = pbpool.tile([128, UBLK * V], bf16, tag="rep", name="rep")
                for uu in range(UBLK):
                    u = g * UBLK + uu
                    ps = psp.tile([128, V], f32, tag="ps", name="ps")
                    for vb in range(NV):
                        sl = slice(vb * 512, (vb + 1) * 512)
                        nc.tensor.matmul(ps[:, sl],
                                         lhsT=sel[:, u * 128:(u + 1) * 128],
                                         rhs=Pb[:, sl],
                                         start=True, stop=True)
                    nc.scalar.copy(rep[:, uu * V:(uu + 1) * V], ps[:, :])
                return rep

            # S3 group 0 via the PE/ACT path (while Pbrep for group 1 builds).
            # DMA in 2-u sub-tiles so output writes start as early as possible.
            rep_next = build_pbrep(1)
            for tb in range(NT):
                ot = opool.tile([128, UBLK * V], bf16, tag="ot", name="ot")
                for uu in range(UBLK):
                    u = uu
                    ps = psp.tile([128, V], f32, tag="ps", name="ps")
                    for vb in range(NV):
                        sl = slice(vb * 512, (vb + 1) * 512)
                        nc.tensor.matmul(ps[:, sl],
                                         lhsT=sel[:, u * 128:(u + 1) * 128],
                                         rhs=Pb[:, sl],
                                         start=True, stop=False)
                        nc.tensor.matmul(ps[:, sl],
                                         lhsT=ident[:, :],
                                         rhs=Ev[tb][:, sl],
                                         start=False, stop=True)
                    nc.scalar.copy(ot[:, uu * V:(uu + 1) * V], ps[:, :])
                    if uu % 2 == 1:
                        nc.sync.dma_start(
                            out_d.ap()[tb * 128:(tb + 1) * 128,
                                       (uu - 1) * V:(uu + 1) * V],
                            ot[:, (uu - 1) * V:(uu + 1) * V])

            # S3 groups 1..NG-1: one FD=4096 DVE tensor_tensor per out tile
            # (in0 = Ev repeated UBLK times via stride-0 broadcast AP).
            for g in range(1, NG):
                rep = rep_next
                if g + 1 < NG:
                    rep_next = build_pbrep(g + 1)
                for tb in range(NT):
                    ot = opool.tile([128, UBLK * V], bf16, tag="ot", name="ot")
                    nc.vector.tensor_tensor(
                        ot[:, :].rearrange("p (r v) -> p r v", v=V),
                        Ev[tb][:, :].unsqueeze(1).broadcast_to((128, UBLK, V)),
                        rep[:, :].rearrange("p (r v) -> p r v", v=V),
                        op=mybir.AluOpType.add)
                    nc.sync.dma_start(
                        out_d.ap()[tb * 128:(tb + 1) * 128,
                                   g * UBLK * V:(g + 1) * UBLK * V],
                        ot[:, :])
    nc.compile()
    return nc


def _get_nc():
    if "nc" not in _CACHE:
        _ensure_path()
        _CACHE["nc"] = _build_nc()
    return _CACHE["nc"]


def _pack(a, nchunks):
    """[nchunks*128, C] -> [128, nchunks*C] (chunk-major along columns)."""
    c = a.shape[1]
    return np.ascontiguousarray(
        a.reshape(nchunks, 128, c).transpose(1, 0, 2).reshape(128, nchunks * c))


def _prep_in_maps(enc_out, pred_out, W_enc, b_enc, W_pred, b_pred, W_out, b_out):
    import ml_dtypes

    f = np.float32
    bf = ml_dtypes.bfloat16
    enc_out = np.asarray(enc_out, f)
    pred_out = np.asarray(pred_out, f)
    W_enc = np.asarray(W_enc, f)
    W_pred = np.asarray(W_pred, f)
    W_out = np.asarray(W_out, f)
    W1, W2 = W_out[:, :J], W_out[:, J:]
    cvec = (W1 @ np.asarray(b_enc, f) + W2 @ np.asarray(b_pred, f)
            + np.asarray(b_out, f)).astype(f)[None, :]
    wce = W1 @ W_enc    # [V, ENC_DIM]
    wcp = W2 @ W_pred   # [V, DEC_DIM]
    shared = {
        "wceT": _pack(np.ascontiguousarray(wce.T), ENC_DIM // 128).astype(bf),
        "wcpT": _pack(np.ascontiguousarray(wcp.T), DEC_DIM // 128).astype(bf),
        "cvec": cvec.astype(bf),
    }
    return [
        {"encT": _pack(np.ascontiguousarray(enc_out[b].T), ENC_DIM // 128).astype(bf),
         "predT": _pack(np.ascontiguousarray(pred_out[b].T), DEC_DIM // 128).astype(bf),
         **shared}
        for b in range(B)
    ]


def run(in_maps, trace=False, **kw):
    _ensure_path()
    from concourse.bass_utils import run_bass_kernel_spmd

    return run_bass_kernel_spmd(_get_nc(), in_maps, list(range(N_CORES)),
                                trace=trace, **kw)


def kernel(enc_out, pred_out, W_enc, b_enc, W_pred, b_pred, W_out, b_out):
    in_maps = _prep_in_maps(enc_out, pred_out, W_enc, b_enc, W_pred, b_pred,
                            W_out, b_out)
    res = run(in_maps, trace=False)
    return np.stack([np.asarray(r["out"]).astype(np.float32).reshape(T, U, V)
                     for r in res.results], axis=0)


# revision 15
# speedup vs baseline: 1.1626x; 1.1626x over previous
"""RNN-T Joiner kernel for 8x TRN2 NeuronCores (Bass/Tile).

out[b,t,u,v] = (enc[b,t]@W_enc.T + b_enc) @ W1.T
            + (pred[b,u]@W_pred.T + b_pred) @ W2.T + b_out
with W1 = W_out[:, :J], W2 = W_out[:, J:].

Strategy: data-parallel over batch (B=8 == n_cores). Host folds the two
back-to-back projections into single matrices (associativity):
  Ev = enc @ (W1@W_enc).T        [T, V]
  Pb = pred @ (W2@W_pred).T + c  [U, V],  c = W1@b_enc + W2@b_pred + b_out
so the device does one GEMM stage instead of two. Per core (one batch):
  S2: Ev (2 t-blocks of 128) and Pb via PE matmuls into PSUM, ACT copies
      to bf16 SBUF. Inputs host-transposed and pre-packed to [128, X] so
      each tensor is a single contiguous DMA.
  S3 (the output): out[t, u, :] = Ev[t, :] + Pb[u, :].
      First UBLK u-values (per t-block) go through the PE: selector-matmul
      broadcasts Pb[u] into PSUM, identity-matmul accumulates Ev, ACT
      copies PSUM -> bf16 out tile. This fills the pipe while Pbrep builds.
      Remaining u: Pb rows are pre-broadcast ("Pbrep" groups of UBLK u,
      double-buffered: sel-matmul -> PSUM -> ACT copy -> bf16 SBUF); the
      add is then ONE pure-SBUF bf16 DVE tensor_tensor per out tile
      (FD=4096, in0 = Ev repeated via a stride-0 broadcast AP, 2x mode).
All matmul operands are bf16 (1 cyc/col on the PE, FWL weight loads);
accumulation stays fp32 in PSUM. Output is written to HBM as bf16 (halves
write traffic; tolerance is 2e-2) and upcast to f32 on host.
"""

import numpy as np

ENC_DIM, DEC_DIM, J, V = 512, 640, 512, 1024
B, T, U = 8, 256, 64
N_CORES = 8
UBLK = 4  # u-values per output tile ([128, UBLK*1024] bf16)
NG = U // UBLK  # 16 u-groups

_CACHE: dict = {}


def _ensure_path():
    try:
        import concourse.bass  # noqa: F401
    except ImportError:
        import sys

        for p in ("/opt/trn_rl_repo", "/root/.axon_site/_ro/trn_rl_repo"):
            if p not in sys.path:
                sys.path.insert(0, p)


def _build_nc():
    import concourse.mybir as mybir
    from concourse import bacc
    from concourse.masks import make_identity
    from concourse.tile import TileContext

    f32 = mybir.dt.float32
    bf16 = mybir.dt.bfloat16
    nc = bacc.Bacc("TRN2", target_bir_lowering=False, debug=False,
                   num_devices=N_CORES)

    NE = ENC_DIM // 128  # 4 contraction chunks for Ev
    ND = DEC_DIM // 128  # 5 contraction chunks for Pb
    NT = T // 128   # 2 t-blocks
    NV = V // 512   # 2 psum-bank v-chunks

    # All inputs host-packed to [128, nchunks*cols] (partition-major chunks).
    encT_d = nc.dram_tensor("encT", [128, NE * T], bf16, kind="ExternalInput")
    predT_d = nc.dram_tensor("predT", [128, ND * U], bf16, kind="ExternalInput")
    wce_d = nc.dram_tensor("wceT", [128, NE * V], bf16, kind="ExternalInput")
    wcp_d = nc.dram_tensor("wcpT", [128, ND * V], bf16, kind="ExternalInput")
    cvec_d = nc.dram_tensor("cvec", [1, V], bf16, kind="ExternalInput")
    out_d = nc.dram_tensor("out", [T, U * V], bf16, kind="ExternalOutput")

    with TileContext(nc) as tc:
        with (
            tc.tile_pool(name="const", bufs=1) as const,
            tc.tile_pool(name="pbrep", bufs=3) as pbpool,
            tc.tile_pool(name="otile", bufs=4) as opool,
            tc.tile_pool(name="ps", bufs=4, space="PSUM") as psp,
        ):
            def load(tag, dram, cols):
                t = const.tile([128, cols], bf16, tag=tag, name=tag)
                nc.sync.dma_start(t[:, :], dram.ap()[:, :])
                return t

            preds_a = load("pred", predT_d, ND * U)
            wcp_a = load("wcp", wcp_d, ND * V)
            encs_a = load("enc", encT_d, NE * T)
            wce_a = load("wce", wce_d, NE * V)
            cvec = const.tile([1, V], bf16, tag="cvec", name="cvec")
            nc.sync.dma_start(cvec[:, :], cvec_d.ap()[:, :])

            ident = const.tile([128, 128], bf16, tag="ident", name="ident")
            make_identity(nc, ident[:, :])
            ones = const.tile([1, 128], bf16, tag="ones", name="ones")
            nc.gpsimd.memset(ones[:, :], 1.0)
            # sel[k, u*128+m] = 1 if k == u else 0: sel[:, u*128:(u+1)*128] is
            # the lhsT that broadcasts Pb row u across all 128 out partitions.
            sel = const.tile([U, U * 128], bf16, tag="sel", name="sel")
            nc.gpsimd.memset(sel[:, :], 0.0)
            nc.gpsimd.affine_select(
                out=sel[:, :].rearrange("p (u m) -> p u m", m=128),
                in_=sel[:, :].rearrange("p (u m) -> p u m", m=128),
                compare_op=mybir.AluOpType.not_equal,
                fill=1.0,
                base=0,
                pattern=[[-1, U], [0, 128]],
                channel_multiplier=1,
            )

            # S2b: Pb = pred @ Wcp.T + c -> [64, 1024]  (c via K=1 matmul)
            Pb = const.tile([U, V], bf16, tag="Pb", name="Pb")
            for vb in range(NV):
                ps = psp.tile([128, V], f32, tag="ps", name="ps")
                for c in range(ND):
                    nc.tensor.matmul(ps[:U, :512],
                                     lhsT=preds_a[:, c * U:(c + 1) * U],
                                     rhs=wcp_a[:, c * V + vb * 512:c * V + (vb + 1) * 512],
                                     start=(c == 0), stop=False)
                nc.tensor.matmul(ps[:U, :512], lhsT=ones[:, :U],
                                 rhs=cvec[:, vb * 512:(vb + 1) * 512],
                                 start=False, stop=True)
                nc.scalar.copy(Pb[:, vb * 512:(vb + 1) * 512], ps[:U, :512])

            # S2a: Ev[tb] = enc @ Wce.T -> [128, 1024] per t-block
            Ev = [const.tile([128, V], bf16, tag=f"Ev{tb}", name=f"Ev{tb}") for tb in range(NT)]
            for tb in range(NT):
                for vb in range(NV):
                    ps = psp.tile([128, V], f32, tag="ps", name="ps")
                    for c in range(NE):
                        nc.tensor.matmul(
                            ps[:, :512],
                            lhsT=encs_a[:, c * T + tb * 128:c * T + (tb + 1) * 128],
                            rhs=wce_a[:, c * V + vb * 512:c * V + (vb + 1) * 512],
                            start=(c == 0), stop=(c == NE - 1))
                    nc.scalar.copy(Ev[tb][:, vb * 512:(vb + 1) * 512], ps[:, :512])

            def build_pbrep(g):
                """Pre-broadcast Pb rows u=g*UBLK..g*UBLK+UBLK-1 across all
                128 partitions: [128, UBLK*1024] bf16."""
                rep = pbpool.tile([128, UBLK * V], bf16, tag="rep", name="rep")
                for uu in range(UBLK):
                    u = g * UBLK + uu
                    ps = psp.tile([128, V], f32, tag="ps", name="ps")
                    for vb in range(NV):
                        sl = slice(vb * 512, (vb + 1) * 512)
                        nc.tensor.matmul(ps[:, sl],
                                         lhsT=sel[:, u * 128:(u + 1) * 128],
                                         rhs=Pb[:, sl],
                                         start=True, stop=True)
                    nc.scalar.copy(rep[:, uu * V:(uu + 1) * V], ps[:, :])
                return rep

            # S3 group 0 via the PE/ACT path (while Pbrep for group 1 builds).
            # DMA in 2-u sub-tiles so output writes start as early as possible.
            rep_next = build_pbrep(1)
            for tb in range(NT):
                ot = opool.tile([128, UBLK * V], bf16, tag="ot", name="ot")
                for uu in range(UBLK):
                    u = uu
                    ps = psp.tile([128, V], f32, tag="ps", name="ps")
                    for vb in range(NV):
                        sl = slice(vb * 512, (vb + 1) * 512)
                        nc.tensor.matmul(ps[:, sl],
                                         lhsT=sel[:, u * 128:(u + 1) * 128],
                                         rhs=Pb[:, sl],
                                         start=True, stop=False)
                        nc.tensor.matmul(ps[:, sl],
                                         lhsT=ident[:, :],
                                         rhs=Ev[tb][:, sl],
                                         start=False, stop=True)
                    nc.scalar.copy(ot[:, uu * V:(uu + 1) * V], ps[:, :])
                    if uu % 2 == 1:
                        nc.sync.dma_start(
                            out_d.ap()[tb * 128:(tb + 1) * 128,
                                       (uu - 1) * V:(uu + 1) * V],
                            ot[:, (uu - 1) * V:(uu + 1) * V])

            # S3 groups 1..NG-1: two FD=2048 DVE tensor_tensors per out tile
            # (in0 = Ev repeated via stride-0 broadcast AP); each TT gates on
            # only half the group's Pbrep copies, and DMAs go out per half.
            H = UBLK // 2
            for g in range(1, NG):
                rep = rep_next
                if g + 1 < NG:
                    rep_next = build_pbrep(g + 1)
                for tb in range(NT):
                    ot = opool.tile([128, UBLK * V], bf16, tag="ot", name="ot")
                    for h in range(2):
                        hs = slice(h * H * V, (h + 1) * H * V)
                        nc.vector.tensor_tensor(
                            ot[:, hs].rearrange("p (r v) -> p r v", v=V),
                            Ev[tb][:, :].unsqueeze(1).broadcast_to((128, H, V)),
                            rep[:, hs].rearrange("p (r v) -> p r v", v=V),
                            op=mybir.AluOpType.add)
                        nc.sync.dma_start(
                            out_d.ap()[tb * 128:(tb + 1) * 128,
                                       (g * UBLK + h * H) * V:
                                       (g * UBLK + (h + 1) * H) * V],
                            ot[:, hs])
    nc.compile()
    return nc


def _get_nc():
    if "nc" not in _CACHE:
        _ensure_path()
        _CACHE["nc"] = _build_nc()
    return _CACHE["nc"]


def _pack(a, nchunks):
    """[nchunks*128, C] -> [128, nchunks*C] (chunk-major along columns)."""
    c = a.shape[1]
    return np.ascontiguousarray(
        a.reshape(nchunks, 128, c).transpose(1, 0, 2).reshape(128, nchunks * c))


def _prep_in_maps(enc_out, pred_out, W_enc, b_enc, W_pred, b_pred, W_out, b_out):
    import ml_dtypes

    f = np.float32
    bf = ml_dtypes.bfloat16
    enc_out = np.asarray(enc_out, f)
    pred_out = np.asarray(pred_out, f)
    W_enc = np.asarray(W_enc, f)
    W_pred = np.asarray(W_pred, f)
    W_out = np.asarray(W_out, f)
    W1, W2 = W_out[:, :J], W_out[:, J:]
    cvec = (W1 @ np.asarray(b_enc, f) + W2 @ np.asarray(b_pred, f)
            + np.asarray(b_out, f)).astype(f)[None, :]
    wce = W1 @ W_enc    # [V, ENC_DIM]
    wcp = W2 @ W_pred   # [V, DEC_DIM]
    shared = {
        "wceT": _pack(np.ascontiguousarray(wce.T), ENC_DIM // 128).astype(bf),
        "wcpT": _pack(np.ascontiguousarray(wcp.T), DEC_DIM // 128).astype(bf),
        "cvec": cvec.astype(bf),
    }
    return [
        {"encT": _pack(np.ascontiguousarray(enc_out[b].T), ENC_DIM // 128).astype(bf),
         "predT": _pack(np.ascontiguousarray(pred_out[b].T), DEC_DIM // 128).astype(bf),
         **shared}
        for b in range(B)
    ]


def run(in_maps, trace=False, **kw):
    _ensure_path()
    from concourse.bass_utils import run_bass_kernel_spmd

    return run_bass_kernel_spmd(_get_nc(), in_maps, list(range(N_CORES)),
                                trace=trace, **kw)


def kernel(enc_out, pred_out, W_enc, b_enc, W_pred, b_pred, W_out, b_out):
    in_maps = _prep_in_maps(enc_out, pred_out, W_enc, b_enc, W_pred, b_pred,
                            W_out, b_out)
    res = run(in_maps, trace=False)
    return np.stack([np.asarray(r["out"]).astype(np.float32).reshape(T, U, V)
                     for r in res.results], axis=0)


# revision 16
# speedup vs baseline: 1.2592x; 1.0832x over previous
"""RNN-T Joiner kernel for 8x TRN2 NeuronCores (Bass/Tile).

out[b,t,u,v] = (enc[b,t]@W_enc.T + b_enc) @ W1.T
            + (pred[b,u]@W_pred.T + b_pred) @ W2.T + b_out
with W1 = W_out[:, :J], W2 = W_out[:, J:].

Strategy: data-parallel over batch (B=8 == n_cores). Host folds the two
back-to-back projections into single matrices (associativity):
  Ev = enc @ (W1@W_enc).T        [T, V]
  Pb = pred @ (W2@W_pred).T + c  [U, V],  c = W1@b_enc + W2@b_pred + b_out
so the device does one GEMM stage. Per core (one batch):
  S2: Ev (both t-blocks into one [128, 2V] tile) and Pb via PE matmuls into
      PSUM, ACT copies to bf16 SBUF. Inputs host-transposed and pre-packed
      to [128, X] so each tensor is a single contiguous DMA.
  S3 (the output): out[t, u, :] = Ev[t, :] + Pb[u, :], processed in units
      of 2 u-values:
      - selector-matmuls broadcast Pb[u0],Pb[u1] (x2 v-halves) into one
        [128, 2048] PSUM tile; one ACT copy -> bf16 "rep" tile (pipelined,
        4-deep pool).
      - ONE DVE tensor_tensor (FD=4096, 2x mode) computes both t-blocks:
        out[p, tb, r, v] = Ev[p, tb*V+v] + rep[p, r*V+v] via stride-0
        broadcast APs; two 0.5MB DMAs (one per t-block) stream it out.
All matmul operands are bf16 (1 cyc/col on the PE, FWL weight loads);
accumulation stays fp32 in PSUM. Output is written to HBM as bf16 (halves
write traffic; tolerance is 2e-2) and upcast to f32 on host.
"""

import numpy as np

ENC_DIM, DEC_DIM, J, V = 512, 640, 512, 1024
B, T, U = 8, 256, 64
N_CORES = 8
NU = U // 2  # 32 2-u units

_CACHE: dict = {}


def _ensure_path():
    try:
        import concourse.bass  # noqa: F401
    except ImportError:
        import sys

        for p in ("/opt/trn_rl_repo", "/root/.axon_site/_ro/trn_rl_repo"):
            if p not in sys.path:
                sys.path.insert(0, p)


def _build_nc():
    import concourse.mybir as mybir
    from concourse import bacc
    from concourse.tile import TileContext

    f32 = mybir.dt.float32
    bf16 = mybir.dt.bfloat16
    nc = bacc.Bacc("TRN2", target_bir_lowering=False, debug=False,
                   num_devices=N_CORES)

    NE = ENC_DIM // 128  # 4 contraction chunks for Ev
    ND = DEC_DIM // 128  # 5 contraction chunks for Pb
    NT = T // 128   # 2 t-blocks

    # All inputs host-packed to [128, nchunks*cols] (partition-major chunks).
    encT_d = nc.dram_tensor("encT", [128, NE * T], bf16, kind="ExternalInput")
    predT_d = nc.dram_tensor("predT", [128, ND * U], bf16, kind="ExternalInput")
    wce_d = nc.dram_tensor("wceT", [128, NE * V], bf16, kind="ExternalInput")
    wcp_d = nc.dram_tensor("wcpT", [128, ND * V], bf16, kind="ExternalInput")
    cvec_d = nc.dram_tensor("cvec", [1, V], bf16, kind="ExternalInput")
    out_d = nc.dram_tensor("out", [T, U * V], bf16, kind="ExternalOutput")

    with TileContext(nc) as tc:
        with (
            tc.tile_pool(name="const", bufs=1) as const,
            tc.tile_pool(name="pbrep", bufs=4) as pbpool,
            tc.tile_pool(name="otile", bufs=4) as opool,
            tc.tile_pool(name="ps", bufs=2, space="PSUM") as psp,
        ):
            def load(tag, dram, cols):
                t = const.tile([128, cols], bf16, tag=tag, name=tag)
                nc.sync.dma_start(t[:, :], dram.ap()[:, :])
                return t

            preds_a = load("pred", predT_d, ND * U)
            wcp_a = load("wcp", wcp_d, ND * V)
            encs_a = load("enc", encT_d, NE * T)
            wce_a = load("wce", wce_d, NE * V)
            cvec = const.tile([1, V], bf16, tag="cvec", name="cvec")
            nc.sync.dma_start(cvec[:, :], cvec_d.ap()[:, :])

            ones = const.tile([1, 128], bf16, tag="ones", name="ones")
            nc.gpsimd.memset(ones[:, :], 1.0)
            # sel[k, u*128+m] = 1 if k == u else 0: sel[:, u*128:(u+1)*128] is
            # the lhsT that broadcasts Pb row u across all 128 out partitions.
            sel = const.tile([U, U * 128], bf16, tag="sel", name="sel")
            nc.gpsimd.memset(sel[:, :], 0.0)
            nc.gpsimd.affine_select(
                out=sel[:, :].rearrange("p (u m) -> p u m", m=128),
                in_=sel[:, :].rearrange("p (u m) -> p u m", m=128),
                compare_op=mybir.AluOpType.not_equal,
                fill=1.0,
                base=0,
                pattern=[[-1, U], [0, 128]],
                channel_multiplier=1,
            )

            # S2b: Pb = pred @ Wcp.T + c -> [64, 1024]  (c via K=1 matmul)
            Pb = const.tile([U, V], bf16, tag="Pb", name="Pb")
            ps = psp.tile([128, 2048], f32, tag="ps", name="ps")
            for vb in range(2):
                for c in range(ND):
                    nc.tensor.matmul(ps[:U, vb * 512:(vb + 1) * 512],
                                     lhsT=preds_a[:, c * U:(c + 1) * U],
                                     rhs=wcp_a[:, c * V + vb * 512:c * V + (vb + 1) * 512],
                                     start=(c == 0), stop=False)
                nc.tensor.matmul(ps[:U, vb * 512:(vb + 1) * 512], lhsT=ones[:, :U],
                                 rhs=cvec[:, vb * 512:(vb + 1) * 512],
                                 start=False, stop=True)
            nc.scalar.copy(Pb[:, :], ps[:U, :V])

            # S2a: Ev (both t-blocks) -> one [128, 2*V] bf16 tile
            Ev = const.tile([128, NT * V], bf16, tag="Ev", name="Ev")
            for tb in range(NT):
                ps = psp.tile([128, 2048], f32, tag="ps", name="ps")
                for vb in range(2):
                    for c in range(NE):
                        nc.tensor.matmul(
                            ps[:, vb * 512:(vb + 1) * 512],
                            lhsT=encs_a[:, c * T + tb * 128:c * T + (tb + 1) * 128],
                            rhs=wce_a[:, c * V + vb * 512:c * V + (vb + 1) * 512],
                            start=(c == 0), stop=(c == NE - 1))
                nc.scalar.copy(Ev[:, tb * V:(tb + 1) * V], ps[:, :V])

            def build_rep(j):
                """Broadcast Pb rows 2j, 2j+1 across partitions -> [128, 2V]."""
                rep = pbpool.tile([128, 2 * V], bf16, tag="rep", name="rep")
                ps = psp.tile([128, 2048], f32, tag="ps", name="ps")
                for r in range(2):
                    u = 2 * j + r
                    for vb in range(2):
                        nc.tensor.matmul(
                            ps[:, r * V + vb * 512:r * V + (vb + 1) * 512],
                            lhsT=sel[:, u * 128:(u + 1) * 128],
                            rhs=Pb[:, vb * 512:(vb + 1) * 512],
                            start=True, stop=True)
                nc.scalar.copy(rep[:, :], ps[:, :])
                return rep

            # S3: per 2-u unit: one FD=4096 DVE TT covers both t-blocks.
            rep_cur = build_rep(0)
            for j in range(NU):
                rep = rep_cur
                if j + 1 < NU:
                    rep_cur = build_rep(j + 1)
                ot = opool.tile([128, 2 * NT * V], bf16, tag="ot", name="ot")
                nc.vector.tensor_tensor(
                    ot[:, :].rearrange("p (tb r v) -> p tb r v", r=2, v=V),
                    Ev[:, :].rearrange("p (tb v) -> p tb v", v=V)
                        .unsqueeze(2).broadcast_to((128, NT, 2, V)),
                    rep[:, :].rearrange("p (r v) -> p r v", v=V)
                        .unsqueeze(1).broadcast_to((128, NT, 2, V)),
                    op=mybir.AluOpType.add)
                for tb in range(NT):
                    nc.sync.dma_start(
                        out_d.ap()[tb * 128:(tb + 1) * 128,
                                   2 * j * V:2 * (j + 1) * V],
                        ot[:, tb * 2 * V:(tb + 1) * 2 * V])
    nc.compile()
    return nc


def _get_nc():
    if "nc" not in _CACHE:
        _ensure_path()
        _CACHE["nc"] = _build_nc()
    return _CACHE["nc"]


def _pack(a, nchunks):
    """[nchunks*128, C] -> [128, nchunks*C] (chunk-major along columns)."""
    c = a.shape[1]
    return np.ascontiguousarray(
        a.reshape(nchunks, 128, c).transpose(1, 0, 2).reshape(128, nchunks * c))


def _prep_in_maps(enc_out, pred_out, W_enc, b_enc, W_pred, b_pred, W_out, b_out):
    import ml_dtypes

    f = np.float32
    bf = ml_dtypes.bfloat16
    enc_out = np.asarray(enc_out, f)
    pred_out = np.asarray(pred_out, f)
    W_enc = np.asarray(W_enc, f)
    W_pred = np.asarray(W_pred, f)
    W_out = np.asarray(W_out, f)
    W1, W2 = W_out[:, :J], W_out[:, J:]
    cvec = (W1 @ np.asarray(b_enc, f) + W2 @ np.asarray(b_pred, f)
            + np.asarray(b_out, f)).astype(f)[None, :]
    wce = W1 @ W_enc    # [V, ENC_DIM]
    wcp = W2 @ W_pred   # [V, DEC_DIM]
    shared = {
        "wceT": _pack(np.ascontiguousarray(wce.T), ENC_DIM // 128).astype(bf),
        "wcpT": _pack(np.ascontiguousarray(wcp.T), DEC_DIM // 128).astype(bf),
        "cvec": cvec.astype(bf),
    }
    return [
        {"encT": _pack(np.ascontiguousarray(enc_out[b].T), ENC_DIM // 128).astype(bf),
         "predT": _pack(np.ascontiguousarray(pred_out[b].T), DEC_DIM // 128).astype(bf),
         **shared}
        for b in range(B)
    ]


def run(in_maps, trace=False, **kw):
    _ensure_path()
    from concourse.bass_utils import run_bass_kernel_spmd

    return run_bass_kernel_spmd(_get_nc(), in_maps, list(range(N_CORES)),
                                trace=trace, **kw)


def kernel(enc_out, pred_out, W_enc, b_enc, W_pred, b_pred, W_out, b_out):
    in_maps = _prep_in_maps(enc_out, pred_out, W_enc, b_enc, W_pred, b_pred,
                            W_out, b_out)
    res = run(in_maps, trace=False)
    return np.stack([np.asarray(r["out"]).astype(np.float32).reshape(T, U, V)
                     for r in res.results], axis=0)
